# revision 1
# baseline (speedup 1.0000x reference)
import sys
sys.path.insert(0, "/opt/trn_rl_repo")
import heapq
import numpy as np
from contextlib import ExitStack

from concourse import bacc, bass, mybir, tile, bass_utils
from concourse.masks import make_identity

P = 128
H = 64
OUT = 32
NB = 4
NCORES = 8
N_DST1 = 100_000
N_DST2 = 20_000
ROWS1 = N_DST1 // NCORES          # 12500 dst1 rows per core
WIN1 = (ROWS1 + P - 1) // P       # 98 windows
BANK = 32768
NBANKS = (1_000_000 + BANK - 1) // BANK   # 31
TOTCOL_CAP = 200
TILES_CAP = 160
BW = 4                            # windows per projection batch
GCAP = 1024                       # max idxs per dma_gather instruction


def _wrap16(a):
    # idx i -> partition i%16, col i//16; replicate x8 to 128 partitions
    n = len(a)
    assert n % 16 == 0
    w = a.reshape(n // 16, 16).T
    return np.tile(w, (8, 1)).astype(np.int16)


def _bin_pack(local_rows, counts, nbins):
    """Assign each row to a bin (<=128 rows/bin), balancing edge load.
    Returns bin_of[i], slot_of[i] aligned with local_rows, and bin loads."""
    order = np.argsort(-counts, kind="stable")
    heap = [(0, b) for b in range(nbins)]
    heapq.heapify(heap)
    nrows = np.zeros(nbins, np.int64)
    load = np.zeros(nbins, np.int64)
    bin_of = np.empty(len(local_rows), np.int64)
    slot_of = np.empty(len(local_rows), np.int64)
    for i in order:
        while True:
            l, b = heapq.heappop(heap)
            if nrows[b] < P:
                break
        bin_of[i] = b
        slot_of[i] = nrows[b]
        nrows[b] += 1
        load[b] += counts[i]
        if nrows[b] < P:
            heapq.heappush(heap, (load[b], b))
    return bin_of, slot_of, load


def _pack_layer(eids_per_core, dst_local_per_core, gidx_per_core, coeff,
                nbins, all_rows=None):
    """Shared packing for both layers. Returns per-core streams + static meta.

    eids: edge ids per core; dst_local: local dst row per edge; gidx: gather
    index per edge (into that core's gather source); coeff: [E,4] global.
    all_rows: if not None (int), every row 0..all_rows-1 gets a bin slot.
    """
    percore = []
    loads_sorted = []
    for c in range(NCORES):
        eids = eids_per_core[c]
        dl = dst_local_per_core[c]
        if all_rows is not None:
            counts = np.bincount(dl, minlength=all_rows)
            rows = np.arange(all_rows)
        else:
            rows, counts = np.unique(dl, return_counts=True)
        bin_of_r, slot_of_r, load = _bin_pack(rows, counts, nbins)
        # relabel bins by load desc so heavy aligns with heavy across cores
        lorder = np.argsort(-load, kind="stable")
        relab = np.empty(nbins, np.int64)
        relab[lorder] = np.arange(nbins)
        bin_of_r = relab[bin_of_r]
        load = load[lorder]
        # per-row lookup
        maxrow = rows.max() + 1 if len(rows) else 1
        row2bin = np.zeros(maxrow, np.int64)
        row2slot = np.zeros(maxrow, np.int64)
        row2bin[rows] = bin_of_r
        row2slot[rows] = slot_of_r
        percore.append(dict(eids=eids, dl=dl, row2bin=row2bin,
                            row2slot=row2slot, rows=rows))
        loads_sorted.append(load)
    loads = np.stack(loads_sorted)                     # [C, nbins]
    T_w = np.maximum(1, -(-loads.max(0) // P))         # tiles per window
    NT = int(T_w.sum())
    # build padded edge streams per core (tile-major)
    streams = []
    for c in range(NCORES):
        d = percore[c]
        eids, dl = d["eids"], d["dl"]
        ebin = d["row2bin"][dl]
        eslot = d["row2slot"][dl]
        g = gidx_per_core[c]
        r_s = np.zeros(NT * P, np.float32)
        c_s = np.zeros((NT * P, NB), np.float32)
        g_s = np.zeros(NT * P, np.int64)
        v_s = np.zeros(NT * P, bool)
        off = 0
        order = np.argsort(ebin * (1 << 40) + g, kind="stable")  # bin, then gidx
        eb_sorted = ebin[order]
        starts = np.searchsorted(eb_sorted, np.arange(nbins))
        ends = np.searchsorted(eb_sorted, np.arange(nbins) + 1)
        for k in range(nbins):
            sel = order[starts[k]:ends[k]]
            n = len(sel)
            cap = int(T_w[k]) * P
            assert n <= cap
            r_s[off:off + n] = eslot[sel]
            c_s[off:off + n] = coeff[eids[sel]]
            g_s[off:off + n] = g[sel]
            v_s[off:off + n] = True
            off += cap
        streams.append(dict(r=r_s, c=c_s, g=g_s, v=v_s))
        d["slot_packed"] = d["row2bin"] * P + d["row2slot"]
    return streams, T_w, NT, percore


def _chunk_windows(T_w, counts_ckb=None, totcol_cap=TOTCOL_CAP,
                   tiles_cap=TILES_CAP):
    """Greedy chunking over window index. counts_ckb: [C, nwin, NBANKS]
    real-edge counts (None -> only tile cap applies)."""
    nwin = len(T_w)
    chunks = []
    k0 = 0
    while k0 < nwin:
        k1 = k0 + 1
        while k1 < nwin:
            nt = int(T_w[k0:k1 + 1].sum())
            ok = nt <= tiles_cap
            if ok and counts_ckb is not None:
                cnt = counts_ckb[:, k0:k1 + 1, :].sum(1).max(0)  # [NBANKS]
                padded = ((cnt + P - 1) // P) * P
                ok = int(padded.sum()) // P <= totcol_cap
            if not ok:
                break
            k1 += 1
        chunks.append((k0, k1))
        k0 = k1
    return chunks


def build(inputs):
    """Host packing + device program. Returns (nc, in_maps, postprocess)."""
    np_in = {k: np.asarray(v) for k, v in inputs.items()}
    input_nodes = np_in["input_nodes"].astype(np.int64)
    src1 = np_in["src1"].astype(np.int64)
    dst1 = np_in["dst1"].astype(np.int64)
    etype1 = np_in["etype1"].astype(np.int64)
    norm1 = np_in["norm1"].astype(np.float32)
    src2 = np_in["src2"].astype(np.int64)
    dst2 = np_in["dst2"].astype(np.int64)
    etype2 = np_in["etype2"].astype(np.int64)
    norm2 = np_in["norm2"].astype(np.float32)
    emb = np.ascontiguousarray(np_in["emb"].astype(np.float32))
    V1 = np_in["V1"].astype(np.float32)
    comp1 = np_in["comp1"].astype(np.float32)
    b1 = np_in["b1"].astype(np.float32)
    V2 = np_in["V2"].astype(np.float32)
    comp2 = np_in["comp2"].astype(np.float32)
    b2 = np_in["b2"].astype(np.float32)

    g1 = input_nodes[src1]
    coeff1 = comp1[etype1] * norm1                     # [E1, 4]
    coeff2 = comp2[etype2] * norm2                     # [E2, 4]
    Vf1 = np.ascontiguousarray(V1.reshape(NB * H, H))
    Vf2 = np.ascontiguousarray(V2.reshape(NB * H, OUT))

    # ---------------- layer 1 packing ----------------
    own1 = dst1 // ROWS1
    e1s = [np.where(own1 == c)[0] for c in range(NCORES)]
    dl1 = [dst1[e] - c * ROWS1 for c, e in enumerate(e1s)]
    gi1 = [g1[e] for c, e in enumerate(e1s)]
    st1, T1, NT1, pc1 = _pack_layer(e1s, dl1, gi1, coeff1, WIN1,
                                    all_rows=ROWS1)

    # per (core, window, bank) real-edge counts for chunking
    cnt_ckb = np.zeros((NCORES, WIN1, NBANKS), np.int64)
    woff = np.zeros(WIN1 + 1, np.int64)
    woff[1:] = np.cumsum(T1) * P
    for c in range(NCORES):
        s = st1[c]
        for k in range(WIN1):
            seg = slice(woff[k], woff[k + 1])
            gv = s["g"][seg][s["v"][seg]]
            cnt_ckb[c, k] = np.bincount(gv >> 15, minlength=NBANKS)
    chunks1 = _chunk_windows(T1, cnt_ckb)

    # chunk meta (static) + per-core G1/G2 index arrays
    chunk_meta = []
    i1g_cols = [[] for _ in range(NCORES)]
    i1e_cols = [[] for _ in range(NCORES)]
    for (k0, k1) in chunks1:
        cnt = cnt_ckb[:, k0:k1, :].sum(1)              # [C, NBANKS]
        padded = (((cnt.max(0) + P - 1) // P) * P).astype(np.int64)
        colbase = np.zeros(NBANKS + 1, np.int64)
        colbase[1:] = np.cumsum(padded // P)
        totcol = int(colbase[-1])
        ntiles = int(T1[k0:k1].sum())
        assert totcol * P <= 32768
        t0 = int(woff[k0]) // P
        chunk_meta.append(dict(k0=k0, k1=k1, padded=padded, colbase=colbase,
                               totcol=totcol, ntiles=ntiles, t0=t0))
        for c in range(NCORES):
            s = st1[c]
            seg = slice(woff[k0], woff[k1])
            g = s["g"][seg]
            v = s["v"][seg]
            bank = (g >> 15)
            flat = np.zeros(len(g), np.int64)
            gblock = np.zeros((P, totcol * 8), np.int16)
            ridx = np.where(v)[0]
            order = ridx[np.argsort(bank[ridx], kind="stable")]
            bo = bank[order]
            bstarts = np.searchsorted(bo, np.arange(NBANKS))
            bends = np.searchsorted(bo, np.arange(NBANKS) + 1)
            for b in range(NBANKS):
                pb = int(padded[b])
                if pb == 0:
                    continue
                sel = order[bstarts[b]:bends[b]]
                pos = np.arange(len(sel))
                flat[sel] = (pos % P) * totcol + colbase[b] + pos // P
                rel = np.zeros(pb, np.int64)
                rel[:len(sel)] = g[sel] & (BANK - 1)
                c0 = int(colbase[b]) * 8
                gblock[:, c0:c0 + pb // 16] = _wrap16(rel)
            i1g_cols[c].append(gblock)
            i1e_cols[c].append(_wrap16(flat))
    IC1 = sum(m["totcol"] * 8 for m in chunk_meta)

    # ---------------- layer 2 packing ----------------
    own2 = src2 // ROWS1
    e2s = [np.where(own2 == c)[0] for c in range(NCORES)]
    dl2 = [dst2[e] for e in e2s]                       # global dst2 as "local"
    gi2 = [pc1[c]["slot_packed"][src2[e] - c * ROWS1] for c, e in enumerate(e2s)]
    W2 = max(-(-len(np.unique(d)) // P) for d in dl2)
    st2, T2, NT2, pc2 = _pack_layer(e2s, dl2, gi2, coeff2, W2)
    woff2 = np.zeros(W2 + 1, np.int64)
    woff2[1:] = np.cumsum(T2) * P
    chunks2 = _chunk_windows(T2)
    chunk2_meta = [dict(k0=k0, k1=k1, t0=int(woff2[k0]) // P,
                        ntiles=int(T2[k0:k1].sum())) for (k0, k1) in chunks2]

    i2g = []
    colids = []
    for c in range(NCORES):
        s = st2[c]
        g = s["g"].copy()
        g[~s["v"]] = 0
        i2g.append(_wrap16(g))
        ids = np.full(W2 * P, -1, np.int64)
        d = pc2[c]
        rows = d["rows"]
        ids[d["row2bin"][rows] * P + d["row2slot"][rows]] = rows
        colids.append(ids)

    # ---------------- device program ----------------
    nc = bacc.Bacc("TRN2", target_bir_lowering=False, debug=False,
                   num_devices=NCORES)
    f32, bf16, i16, i32 = (mybir.dt.float32, mybir.dt.bfloat16,
                           mybir.dt.int16, mybir.dt.int32)
    emb_d = nc.dram_tensor("emb", [1_000_000, H], f32, kind="ExternalInput").ap()
    vf1_d = nc.dram_tensor("vf1", [NB * H, H], f32, kind="ExternalInput").ap()
    vf2_d = nc.dram_tensor("vf2", [NB * H, OUT], f32, kind="ExternalInput").ap()
    b1_d = nc.dram_tensor("b1v", [H], f32, kind="ExternalInput").ap()
    r1_d = nc.dram_tensor("r1", [P, NT1], f32, kind="ExternalInput").ap()
    c1_d = nc.dram_tensor("c1", [P, NT1, NB], f32, kind="ExternalInput").ap()
    i1g_d = nc.dram_tensor("i1g", [P, IC1], i16, kind="ExternalInput").ap()
    i1e_d = nc.dram_tensor("i1e", [P, NT1 * 8], i16, kind="ExternalInput").ap()
    r2_d = nc.dram_tensor("r2", [P, NT2], f32, kind="ExternalInput").ap()
    c2_d = nc.dram_tensor("c2", [P, NT2, NB], f32, kind="ExternalInput").ap()
    i2g_d = nc.dram_tensor("i2g", [P, NT2 * 8], i16, kind="ExternalInput").ap()
    xe_ds = [nc.dram_tensor(f"xe{j}", [P, m["totcol"], H], f32, kind="Internal").ap()
             for j, m in enumerate(chunk_meta)]
    h1_d = nc.dram_tensor("h1", [WIN1 * P, H], f32, kind="Internal").ap()
    h2_d = nc.dram_tensor("h2", [OUT, W2 * P], f32, kind="ExternalOutput").ap()

    TOTCOL_MAX = max(m["totcol"] for m in chunk_meta)
    NTC_MAX = max(m["ntiles"] for m in chunk_meta)
    NTC2_MAX = max(m["ntiles"] for m in chunk2_meta)

    with tile.TileContext(nc) as tc:
        with ExitStack() as pctx:
            pp = pctx.enter_context(tc.tile_pool(name="pp", bufs=1))
            ppa = pctx.enter_context(tc.tile_pool(name="ppa", bufs=2, space="PSUM"))
            pph = pctx.enter_context(tc.tile_pool(name="pph", bufs=2, space="PSUM"))
            ppt = pctx.enter_context(tc.tile_pool(name="ppt", bufs=2, space="PSUM"))

            vf1_f = pp.tile([P, 2, H], f32)
            vf1_t = pp.tile([P, 2, H], bf16)
            vf2_f = pp.tile([P, 2, OUT], f32)
            vf2_t = pp.tile([P, 2, OUT], bf16)
            b1_t = pp.tile([H, 1], f32)
            iota_i = pp.tile([P, P], i32)
            iota_f = pp.tile([P, P], f32)
            ident = pp.tile([P, P], f32)
            nc.sync.dma_start(vf1_f[:, 0, :], vf1_d[0:P, :])
            nc.sync.dma_start(vf1_f[:, 1, :], vf1_d[P:2 * P, :])
            nc.sync.dma_start(vf2_f[:, 0, :], vf2_d[0:P, :])
            nc.sync.dma_start(vf2_f[:, 1, :], vf2_d[P:2 * P, :])
            nc.sync.dma_start(b1_t[:], b1_d[:, None])
            nc.vector.tensor_copy(vf1_t[:], vf1_f[:])
            nc.vector.tensor_copy(vf2_t[:], vf2_f[:])
            nc.gpsimd.iota(iota_i[:], pattern=[[1, P]], base=0, channel_multiplier=0)
            nc.scalar.copy(iota_f[:], iota_i[:])
            make_identity(nc, ident[:])

            def do_windows(tc_pool, psum_a, psum_h, xbuf, t0_glob, t0_loc,
                           krange, T_arr, r_t, c_t, vf_t, nout, is_l1):
                """Emit compute for windows krange (list of global window ids)."""
                wlist = list(krange)
                for gstart in range(0, len(wlist), BW):
                    gwin = wlist[gstart:gstart + BW]
                    bw = len(gwin)
                    Ab0 = tc_pool.tile([P, bw, P], bf16)
                    Ab1 = tc_pool.tile([P, bw, P], bf16)
                    for wi, k in enumerate(gwin):
                        A0 = psum_a.tile([P, P], f32)
                        A1 = psum_a.tile([P, P], f32)
                        Tk = int(T_arr[k])
                        tbase = int(np.sum(T_arr[:k]))
                        for j in range(Tk):
                            t = tbase + j
                            tloc = t - t0_loc
                            K_t = tc_pool.tile([P, NB * H], bf16)
                            S_t = tc_pool.tile([P, P], bf16)
                            for b in range(NB):
                                if b % 2 == 0:
                                    nc.scalar.mul(K_t[:, b * H:(b + 1) * H],
                                                  xbuf[:, tloc, :],
                                                  c_t[:, t, b:b + 1])
                                else:
                                    nc.vector.tensor_scalar(
                                        out=K_t[:, b * H:(b + 1) * H],
                                        in0=xbuf[:, tloc, :],
                                        scalar1=c_t[:, t, b:b + 1],
                                        scalar2=None, op0=mybir.AluOpType.mult)
                            nc.vector.tensor_tensor(
                                out=S_t[:], in0=r_t[:, t:t + 1].to_broadcast([P, P]),
                                in1=iota_f[:], op=mybir.AluOpType.is_equal)
                            nc.tensor.matmul(out=A0[:], lhsT=K_t[:, 0:P],
                                             rhs=S_t[:], start=(j == 0),
                                             stop=(j == Tk - 1))
                            nc.tensor.matmul(out=A1[:], lhsT=K_t[:, P:2 * P],
                                             rhs=S_t[:], start=(j == 0),
                                             stop=(j == Tk - 1))
                        nc.scalar.copy(Ab0[:, wi, :], A0[:])
                        nc.scalar.copy(Ab1[:, wi, :], A1[:])
                    hT_ps = psum_h.tile([nout, bw * P], f32)
                    nc.tensor.matmul(out=hT_ps[:], lhsT=vf_t[:, 0, :],
                                     rhs=Ab0[:].rearrange("p a b -> p (a b)"),
                                     start=True, stop=False)
                    nc.tensor.matmul(out=hT_ps[:], lhsT=vf_t[:, 1, :],
                                     rhs=Ab1[:].rearrange("p a b -> p (a b)"),
                                     start=False, stop=True)
                    hT_sb = tc_pool.tile([nout, bw * P], f32)
                    if is_l1:
                        nc.scalar.activation(out=hT_sb[:], in_=hT_ps[:],
                                             func=mybir.ActivationFunctionType.Relu,
                                             bias=b1_t[:, 0:1])
                        for wi, k in enumerate(gwin):
                            h_ps = ppt.tile([P, H], f32)
                            h_sb = tc_pool.tile([P, H], f32)
                            nc.tensor.transpose(h_ps[:],
                                                hT_sb[:, wi * P:(wi + 1) * P],
                                                ident[0:H, 0:H])
                            nc.scalar.copy(h_sb[:], h_ps[:])
                            nc.sync.dma_start(h1_d[k * P:(k + 1) * P, :], h_sb[:])
                    else:
                        nc.scalar.copy(hT_sb[:], hT_ps[:])
                        k0 = gwin[0]
                        nc.sync.dma_start(h2_d[:, k0 * P:k0 * P + bw * P], hT_sb[:])

            # -------- layer 1 --------
            with ExitStack() as l1ctx:
                pd = l1ctx.enter_context(tc.tile_pool(name="pd", bufs=1))
                pg = l1ctx.enter_context(tc.tile_pool(name="pg", bufs=1))
                px = l1ctx.enter_context(tc.tile_pool(name="px", bufs=2))
                pk = l1ctx.enter_context(tc.tile_pool(name="pk", bufs=3))
                r1_t = pd.tile([P, NT1], f32)
                c1_t = pd.tile([P, NT1, NB], f32)
                i1g_t = pd.tile([P, IC1], i16)
                i1e_t = pd.tile([P, NT1 * 8], i16)
                nc.sync.dma_start(r1_t[:], r1_d[:])
                nc.sync.dma_start(c1_t[:], c1_d[:])
                nc.sync.dma_start(i1g_t[:], i1g_d[:])
                nc.sync.dma_start(i1e_t[:], i1e_d[:])

                gcol0 = 0
                for j, m in enumerate(chunk_meta):
                    totcol, ntiles, t0 = m["totcol"], m["ntiles"], m["t0"]
                    gbuf = pg.tile([P, TOTCOL_MAX, H], f32)
                    for b in range(NBANKS):
                        pb = int(m["padded"][b])
                        if pb == 0:
                            continue
                        cb = int(m["colbase"][b])
                        lo = b * BANK
                        hi = min(lo + BANK, 1_000_000)
                        for o in range(0, pb, GCAP):
                            n = min(GCAP, pb - o)
                            nc.gpsimd.dma_gather(
                                out_ap=gbuf[:, cb + o // P:cb + (o + n) // P, :],
                                in_ap=emb_d[lo:hi, :],
                                idxs_ap=i1g_t[:, gcol0 + cb * 8 + o // 16:
                                              gcol0 + cb * 8 + (o + n) // 16],
                                num_idxs=n, num_idxs_reg=n, elem_size=H)
                    nc.sync.dma_start(xe_ds[j][:], gbuf[:, 0:totcol, :])
                    xbuf = px.tile([P, NTC_MAX, H], f32)
                    for o in range(0, ntiles * P, GCAP):
                        n = min(GCAP, ntiles * P - o)
                        nc.gpsimd.dma_gather(
                            out_ap=xbuf[:, o // P:(o + n) // P, :],
                            in_ap=xe_ds[j].rearrange("p t d -> (p t) d"),
                            idxs_ap=i1e_t[:, t0 * 8 + o // 16:t0 * 8 + (o + n) // 16],
                            num_idxs=n, num_idxs_reg=n, elem_size=H)
                    do_windows(pk, ppa, pph, xbuf, None, t0,
                               range(m["k0"], m["k1"]), T1, r1_t, c1_t,
                               vf1_t, H, True)
                    gcol0 += totcol * 8

            # -------- layer 2 --------
            with ExitStack() as l2ctx:
                pd2 = l2ctx.enter_context(tc.tile_pool(name="pd2", bufs=1))
                px2 = l2ctx.enter_context(tc.tile_pool(name="px2", bufs=2))
                pk2 = l2ctx.enter_context(tc.tile_pool(name="pk2", bufs=3))
                r2_t = pd2.tile([P, NT2], f32)
                c2_t = pd2.tile([P, NT2, NB], f32)
                i2g_t = pd2.tile([P, NT2 * 8], i16)
                nc.sync.dma_start(r2_t[:], r2_d[:])
                nc.sync.dma_start(c2_t[:], c2_d[:])
                nc.sync.dma_start(i2g_t[:], i2g_d[:])
                for m in chunk2_meta:
                    ntiles, t0 = m["ntiles"], m["t0"]
                    xbuf2 = px2.tile([P, NTC2_MAX, H], f32)
                    for o in range(0, ntiles * P, GCAP):
                        n = min(GCAP, ntiles * P - o)
                        nc.gpsimd.dma_gather(
                            out_ap=xbuf2[:, o // P:(o + n) // P, :],
                            in_ap=h1_d[:],
                            idxs_ap=i2g_t[:, t0 * 8 + o // 16:t0 * 8 + (o + n) // 16],
                            num_idxs=n, num_idxs_reg=n, elem_size=H)
                    do_windows(pk2, ppa, pph, xbuf2, None, t0,
                               range(m["k0"], m["k1"]), T2, r2_t, c2_t,
                               vf2_t, OUT, False)

    nc.compile()

    in_maps = []
    for c in range(NCORES):
        s1, s2 = st1[c], st2[c]
        in_maps.append({
            "emb": emb, "vf1": Vf1, "vf2": Vf2, "b1v": b1,
            "r1": np.ascontiguousarray(s1["r"].reshape(NT1, P).T),
            "c1": np.ascontiguousarray(
                s1["c"].reshape(NT1, P, NB).transpose(1, 0, 2)),
            "i1g": np.concatenate(i1g_cols[c], axis=1),
            "i1e": np.concatenate(i1e_cols[c], axis=1),
            "r2": np.ascontiguousarray(s2["r"].reshape(NT2, P).T),
            "c2": np.ascontiguousarray(
                s2["c"].reshape(NT2, P, NB).transpose(1, 0, 2)),
            "i2g": i2g[c],
        })

    def post(results):
        out = np.zeros((N_DST2, OUT), np.float32)
        for c in range(NCORES):
            h2 = np.asarray(results[c]["h2"])          # [OUT, W2*P]
            ids = colids[c]
            v = ids >= 0
            out[ids[v]] += h2.T[v]
        out += b2[None, :]
        return out

    return nc, in_maps, post


def kernel(**inputs):
    nc, in_maps, post = build(inputs)
    res = bass_utils.run_bass_kernel_spmd(nc, in_maps, list(range(NCORES)))
    return post(res.results)

